# revision 1
# baseline (speedup 1.0000x reference)
"""Causal single-head attention on 8 trn2 NeuronCores.

Problem: x[4, 2048, 1024] fp32, W_q/W_k/W_v [1024, 1024] fp32 (torch Linear
layout, y = x @ W.T). Causal softmax attention, d_out = 1024.

Sharding: data-parallel over batch (4) x 2-way KEY split per batch.
Core c = 2*b + h handles batch b and the eight 128-row KEY blocks
{h, h+2, ..., h+14} (global 128-row block indices). Each core computes
Q for ALL 2048 queries but K/V only for its 1024 keys, runs flash-style
attention of all queries against its keys (exp without max subtraction —
scores are bounded — so partial sums merge exactly), and outputs the
unnormalized partial AV (bf16) and partial denominator (fp32). The host
merges: out = (AV_0 + AV_1) / (den_0 + den_1). Duplicating only Q (one
tensor) instead of K and V (two) minimizes the no-communication
projection cost, and interleaving key blocks at 128-row granularity makes
the local causal extent for query superblock m exactly 128*(m+1) on BOTH
cores of a pair — one SPMD program, zero static extent waste at tile
granularity, and only the last local key tile of each superblock needs a
causal mask, which is a single m-independent [128, 256] additive tile
whose data depends only on h (supplied as input).

Per-core device program (matmul operands bf16, fp32 PSUM accumulate):
  1. Projections: V[skl, o], K^T[o, skl] from gathered-key x^T; Q^T[o, sq]
     from full x^T. A short dependency-free matmul warmup keeps the PE
     clock un-gated while the first DMAs land.
  2. Attention, key-stationary scores pass: per local key tile t, scores
     S^T[128, w] against all query superblocks m >= t in 512-wide matmul
     chunks, diagonal mask add, exp (ACT, scale 1/32) into resident bf16
     es tiles. Query-stationary AV pass: per superblock m, accumulate AV
     and the softmax denominator (PE ones-matmul) in PSUM over tiles
     0..m, evacuate av as bf16 + den as fp32.
"""

import copy

import numpy as np
import ml_dtypes

import concourse.bass as bass
import concourse.mybir as mybir
import concourse.tile as tile
from concourse.bass_utils import run_bass_kernel_spmd

BF16 = mybir.dt.bfloat16
F32 = mybir.dt.float32

B, S, D = 4, 2048, 1024
N_CORES = 8
SB = 256            # query superblock rows / key gather block
N_SB = S // SB      # 8 query superblocks per core
SKL = S // 2        # local keys per core (1024)
MASK_NEG = -1.0e5


def _legalize_waits(nc):
    """Split multi-wait instructions into single-wait NOP chains.

    The walrus here accepts at most one sync-wait command per instruction,
    while TileContext emits several `on_wait` entries on one instruction.
    Hoist all but the last wait onto same-engine NOPs placed immediately
    before the instruction; the engine sequencer stalls on each in order.
    """
    uid = 0
    for fn in nc.m.functions:
        for bb in fn.blocks:
            out = []
            for inst in bb.instructions:
                si = inst.sync_info
                waits = list(si.on_wait) if si and si.on_wait else []
                if len(waits) > 1:
                    for w in waits[:-1]:
                        nop = mybir.InstNoOp(name=f"waitsplit_{uid}", ins=[], outs=[])
                        uid += 1
                        nop.engine = inst.engine
                        si2 = copy.deepcopy(si)
                        si2.on_wait = [w]
                        si2.on_update = []
                        nop.sync_info = si2
                        out.append(nop)
                    si.on_wait = waits[-1:]
                    inst.sync_info = si
                out.append(inst)
            bb.instructions = out


def build_nc(reps=1):
    nc = bass.Bass("TRN2", target_bir_lowering=False, debug=False, num_devices=N_CORES)

    xT_d = nc.dram_tensor("xT", [D, S], BF16, kind="ExternalInput")      # all queries
    xkT_d = nc.dram_tensor("xkT", [D, SKL], BF16, kind="ExternalInput")  # gathered keys
    wqT_d = nc.dram_tensor("wqT", [D, D], BF16, kind="ExternalInput")
    wkT_d = nc.dram_tensor("wkT", [D, D], BF16, kind="ExternalInput")
    wvT_d = nc.dram_tensor("wvT", [D, D], BF16, kind="ExternalInput")
    # additive causal mask for the last 128 local keys of a superblock,
    # [128 keys, 256 queries]; data depends only on h
    mask_d = nc.dram_tensor("maskT", [128, SB], F32, kind="ExternalInput")
    yav_d = nc.dram_tensor("yav", [S, D], BF16, kind="ExternalOutput")
    yden_d = nc.dram_tensor("yden", [S, 1], F32, kind="ExternalOutput")

    ND = D // 128       # 8 d-tiles
    NO = D // 128       # 8 o-tiles
    NSKL = SKL // 128   # 8 local key tiles

    with tile.TileContext(nc) as tc:
        with (
            tc.tile_pool(name="xT", bufs=ND) as xT_pool,
            tc.tile_pool(name="xkT", bufs=ND) as xkT_pool,
            tc.tile_pool(name="w", bufs=2 * ND) as w_pool,
            tc.tile_pool(name="KT", bufs=NO) as KT_pool,
            tc.tile_pool(name="V", bufs=NSKL) as V_pool,
            tc.tile_pool(name="QT", bufs=NO) as QT_pool,
            tc.tile_pool(name="mask", bufs=1) as mask_pool,
            tc.tile_pool(name="ones", bufs=1) as ones_pool,
            tc.tile_pool(name="es", bufs=3) as es_pool,
            tc.tile_pool(name="oav", bufs=6) as oav_pool,
            tc.tile_pool(name="oden", bufs=4) as oden_pool,
            tc.tile_pool(name="warm", bufs=2) as warm_pool,
        ):
            for rep in range(reps):
                # ---- HAM warmup: dependency-free matmuls keep PE busy during
                # the initial DMA wait so phase 1 starts at full clock
                wsrc = warm_pool.tile([128, 512], BF16, tag="wsrc", name=f"wsrc{rep}")
                nc.gpsimd.memset(wsrc[:], 0.0)
                with tc.tile_pool(name="wps", bufs=1, space="PSUM") as wps_pool:
                    wps = wps_pool.tile([128, 512], F32, tag="wps", name=f"wps{rep}")
                    for i in range(19):
                        nc.tensor.matmul(
                            wps[:], wsrc[:, 0:128], wsrc[:], start=(i == 0),
                            stop=(i == 18),
                        )
                # ---- loads, in consumption order: V <- (wv, xkT); KT <- wk; QT <- (xT, wq)
                wv_t = []
                for i in range(ND):
                    t = w_pool.tile([128, D], BF16, tag="w", name=f"wv{i}")
                    nc.sync.dma_start(t[:], wvT_d[i * 128:(i + 1) * 128, :])
                    wv_t.append(t)
                xk_t = []
                for i in range(ND):
                    t = xkT_pool.tile([128, SKL], BF16, tag="xkT", name=f"xk{i}")
                    nc.sync.dma_start(t[:], xkT_d[i * 128:(i + 1) * 128, :])
                    xk_t.append(t)
                wk_t = []
                for i in range(ND):
                    t = w_pool.tile([128, D], BF16, tag="w", name=f"wk{i}")
                    nc.sync.dma_start(t[:], wkT_d[i * 128:(i + 1) * 128, :])
                    wk_t.append(t)
                xT_t = []
                for i in range(ND):
                    t = xT_pool.tile([128, S], BF16, tag="xT", name=f"xq{i}")
                    nc.sync.dma_start(t[:], xT_d[i * 128:(i + 1) * 128, :])
                    xT_t.append(t)
                wq_t = []
                for i in range(ND):
                    t = w_pool.tile([128, D], BF16, tag="w", name=f"wq{i}")
                    nc.sync.dma_start(t[:], wqT_d[i * 128:(i + 1) * 128, :])
                    wq_t.append(t)
                mask_t = []
                for i in range(1):
                    t = mask_pool.tile([128, SB], F32, tag="mask", name=f"mask{i}")
                    nc.sync.dma_start(t[:], mask_d[i * 128:(i + 1) * 128, :])
                    mask_t.append(t)
                ones_t = ones_pool.tile([128, 1], BF16)
                nc.gpsimd.memset(ones_t[:], 1.0)

                # ---- phase 1: projections ----
                with tc.tile_pool(name="psum1", bufs=4, space="PSUM") as psum1:
                    # V[skl, o] = sum_d xkT[d, skl] * wvT[d, o]
                    V_t = [
                        V_pool.tile([128, D], BF16, tag="V", name=f"V{st}") for st in range(NSKL)
                    ]
                    for st in range(NSKL):
                        for oc in range(D // 512):
                            ps = psum1.tile([128, 512], F32, tag="ps1", name=f"psv{st}_{oc}")
                            for d in range(ND):
                                nc.tensor.matmul(
                                    ps[:],
                                    xk_t[d][:, st * 128:(st + 1) * 128],
                                    wv_t[d][:, oc * 512:(oc + 1) * 512],
                                    start=(d == 0),
                                    stop=(d == ND - 1),
                                )
                            nc.vector.tensor_copy(V_t[st][:, oc * 512:(oc + 1) * 512], ps[:])
                    # K^T[o, skl] = sum_d wkT[d, o] * xkT[d, skl]
                    KT_t = [
                        KT_pool.tile([128, SKL], BF16, tag="KT", name=f"KT{ot}") for ot in range(NO)
                    ]
                    for skc in range(SKL // 512):
                        for ot in range(NO):
                            ps = psum1.tile([128, 512], F32, tag="ps1", name=f"psk{skc}_{ot}")
                            for d in range(ND):
                                nc.tensor.matmul(
                                    ps[:],
                                    wk_t[d][:, ot * 128:(ot + 1) * 128],
                                    xk_t[d][:, skc * 512:(skc + 1) * 512],
                                    start=(d == 0),
                                    stop=(d == ND - 1),
                                )
                            nc.vector.tensor_copy(
                                KT_t[ot][:, skc * 512:(skc + 1) * 512], ps[:]
                            )
                    # Q^T[o, sq] = sum_d wqT[d, o] * xT[d, sq]   (all 2048 queries)
                    QT_t = [
                        QT_pool.tile([128, S], BF16, tag="QT", name=f"QT{ot}") for ot in range(NO)
                    ]
                    for sqc in range(S // 512):
                        for ot in range(NO):
                            ps = psum1.tile([128, 512], F32, tag="ps1", name=f"psq{sqc}_{ot}")
                            for d in range(ND):
                                nc.tensor.matmul(
                                    ps[:],
                                    wq_t[d][:, ot * 128:(ot + 1) * 128],
                                    xT_t[d][:, sqc * 512:(sqc + 1) * 512],
                                    start=(d == 0),
                                    stop=(d == ND - 1),
                                )
                            nc.vector.tensor_copy(
                                QT_t[ot][:, sqc * 512:(sqc + 1) * 512], ps[:]
                            )

                # ---- phase 2: attention (partial over local keys) ----
                # Pass A (key-stationary): for each local key tile t, scores
                # against ALL query superblocks m >= t in 512-wide chunks,
                # mask + exp into resident es tiles. Pass B (query-stationary):
                # per superblock, accumulate AV and denominator in PSUM.
                with (
                    tc.tile_pool(name="av", bufs=4, space="PSUM") as av_pool,
                    tc.tile_pool(name="pss", bufs=2, space="PSUM") as pss_pool,
                    tc.tile_pool(name="den", bufs=2, space="PSUM") as den_pool,
                ):
                    es_t = []
                    for t in range(NSKL):
                        w = S - SB * t          # queries [SB*t, S)
                        es = es_pool.tile([128, w], BF16, tag=f"es{t}", bufs=1,
                                          name=f"es{rep}_{t}")
                        es_t.append(es)
                        for c0 in range(0, w, 512):
                            cw = min(512, w - c0)
                            ps_s = pss_pool.tile([128, 512], F32, tag="pss",
                                                 name=f"pss{t}_{c0}")
                            for od in range(NO):
                                nc.tensor.matmul(
                                    ps_s[:, 0:cw],
                                    KT_t[od][:, t * 128:(t + 1) * 128],
                                    QT_t[od][:, SB * t + c0:SB * t + c0 + cw],
                                    start=(od == 0),
                                    stop=(od == NO - 1),
                                )
                            if c0 == 0:
                                # first 256 queries belong to superblock m=t:
                                # causal boundary mask for the diagonal tile
                                nc.vector.tensor_add(
                                    ps_s[:, 0:SB], ps_s[:, 0:SB], mask_t[0][:]
                                )
                            nc.scalar.activation(
                                es[:, c0:c0 + cw], ps_s[:, 0:cw],
                                mybir.ActivationFunctionType.Exp,
                                scale=1.0 / 32.0,
                            )
                    for m in range(N_SB):
                        n = m + 1              # local sk-tiles this superblock
                        avs = [
                            av_pool.tile([128, 512], F32, tag="av", name=f"av{m}_{i}")
                            for i in range(4)
                        ]
                        dens = [
                            den_pool.tile([128, 1], F32, tag="den", name=f"den{m}_{i}")
                            for i in range(2)
                        ]
                        # tile-major: finish each av tile's accumulation
                        # before starting the next, so its evacuation overlaps
                        # the remaining AV matmuls instead of serializing after
                        # the superblock's last matmul
                        for sqh in range(2):
                            for t in range(n):
                                q0 = SB * (m - t)
                                nc.tensor.matmul(
                                    dens[sqh][:],
                                    es_t[t][:, q0 + sqh * 128:q0 + (sqh + 1) * 128],
                                    ones_t[:],
                                    start=(t == 0),
                                    stop=(t == n - 1),
                                )
                        for sqh in range(2):
                            for oh in range(2):
                                for t in range(n):
                                    q0 = SB * (m - t)
                                    nc.tensor.matmul(
                                        avs[sqh * 2 + oh][:],
                                        es_t[t][:, q0 + sqh * 128:q0 + (sqh + 1) * 128],
                                        V_t[t][:, oh * 512:(oh + 1) * 512],
                                        start=(t == 0),
                                        stop=(t == n - 1),
                                    )
                        for sqh in range(2):
                            r0 = m * SB + sqh * 128
                            oden = oden_pool.tile([128, 1], F32, tag="oden", name=f"oden{m}_{sqh}")
                            nc.vector.tensor_copy(oden[:], dens[sqh][:])
                            nc.sync.dma_start(yden_d[r0:r0 + 128, :], oden[:])
                            oav = oav_pool.tile([128, D], BF16, tag="oav", name=f"oav{m}_{sqh}")
                            for oh in range(2):
                                nc.vector.tensor_copy(
                                    oav[:, oh * 512:(oh + 1) * 512], avs[sqh * 2 + oh][:]
                                )
                                if m == N_SB - 1:
                                    # kernel tail: ship each half as soon as
                                    # its copy lands
                                    nc.sync.dma_start(
                                        yav_d[r0:r0 + 128, oh * 512:(oh + 1) * 512],
                                        oav[:, oh * 512:(oh + 1) * 512],
                                    )
                            if m != N_SB - 1:
                                nc.sync.dma_start(yav_d[r0:r0 + 128, :], oav[:])

    _legalize_waits(nc)
    return nc


_NC_CACHE = None


def _get_nc():
    global _NC_CACHE
    if _NC_CACHE is None:
        _NC_CACHE = build_nc()
    return _NC_CACHE


def _prep_core_inputs(x, wqT, wkT, wvT, b, h):
    xb = np.ascontiguousarray(x[b])                       # [S, D] fp32
    xT = np.ascontiguousarray(xb.T).astype(ml_dtypes.bfloat16)
    # keys interleave at 128-row granularity: core h owns global 128-blocks
    # {h, h+2, ..., h+14}; superblock m's local extent is exactly 128*(m+1)
    # for both cores, and only the last local key tile needs masking.
    blocks = [h + 2 * i for i in range(8)]
    xk = np.concatenate([xb[128 * t:128 * (t + 1)] for t in blocks], axis=0)
    xkT = np.ascontiguousarray(xk.T).astype(ml_dtypes.bfloat16)
    # mask for the last local key tile (global block 2m+h vs queries of
    # superblock m): keep iff 128*h + r_k <= r_q
    kk = np.arange(128)[:, None]
    qq = np.arange(SB)[None, :]
    maskT = np.where(128 * h + kk <= qq, 0.0, MASK_NEG).astype(np.float32)
    return {
        "xT": xT, "xkT": xkT,
        "wqT": wqT, "wkT": wkT, "wvT": wvT,
        "maskT": maskT,
    }


def kernel(x, W_q, W_k, W_v):
    x = np.asarray(x, dtype=np.float32)
    wqT = np.ascontiguousarray(np.asarray(W_q, np.float32).T).astype(ml_dtypes.bfloat16)
    wkT = np.ascontiguousarray(np.asarray(W_k, np.float32).T).astype(ml_dtypes.bfloat16)
    wvT = np.ascontiguousarray(np.asarray(W_v, np.float32).T).astype(ml_dtypes.bfloat16)

    in_maps = []
    for c in range(N_CORES):
        b, h = divmod(c, 2)
        in_maps.append(_prep_core_inputs(x, wqT, wkT, wvT, b, h))

    nc = _get_nc()
    res = run_bass_kernel_spmd(nc, in_maps, list(range(N_CORES)))

    out = np.empty((B, S, D), dtype=np.float32)
    for b in range(B):
        av0 = np.asarray(res.results[2 * b]["yav"], dtype=np.float32)
        av1 = np.asarray(res.results[2 * b + 1]["yav"], dtype=np.float32)
        den = res.results[2 * b]["yden"] + res.results[2 * b + 1]["yden"]
        out[b] = (av0 + av1) / den
    return out



# revision 4
# speedup vs baseline: 2.2241x; 2.2241x over previous
"""Causal single-head attention on 8 trn2 NeuronCores.

Problem: x[4, 2048, 1024] fp32, W_q/W_k/W_v [1024, 1024] fp32 (torch Linear
layout, y = x @ W.T). Causal softmax attention, d_out = 1024.

Sharding: data-parallel over batch (4) x 2-way KEY split per batch, with the
Q projection split across the pair and exchanged on-chip. Core c = 2*b + h
handles batch b and the eight 128-row KEY blocks {h, h+2, ..., h+14}. Each
core projects Q ONLY for its own half of the queries [1024*h, 1024*h+1024)
(the baseline duplicated the full-Q projection on both cores of a pair --
2.1 GMAC of redundant PE work per core), plus K/V for its local keys. The
halves are exchanged with a single 8-core AllGather into a Shared DRAM
buffer; each core then pulls exactly its pair's two slabs back with an
indirect (index-driven) DMA gather, whose index data is a per-core input --
so one SPMD program serves all cores despite the core-dependent slab offset.

Layout trick: the gather view of the gathered buffer is [1024 rows, 8192],
one row = 16KB = 8 projection rows. Staging the Q^T chunk with a permuted
row order (PSUM partition p of o-tile ot -> row 8p + ot, i.e. row r holds
o-dim 128*(r%8) + r//8) makes the gathered SBUF tile QG[p, 1024*j + q] =
Q^T[128*j + p, q]: natural contiguous d-blocks per 1024-column group, so
the scores matmuls pair it with the baseline's natural K^T tiles.

Attention (per core, matmul operands bf16, fp32 PSUM accumulate): flash
style without max-subtraction (scores bounded), key-stationary scores pass
producing resident exp-score tiles, then query-stationary AV + denominator
pass; host merges the pair's unnormalized AV partials and denominators.
The scores pass runs query-chunk-major (512-wide chunks ascending) so the
first chunks only need the first gathered slab. Warmup matmuls and the
(identical every rep) weight loads are emitted only for rep 0.
"""

import copy

import numpy as np
import ml_dtypes

import concourse.bass as bass
import concourse.mybir as mybir
import concourse.tile as tile
from concourse.bass_utils import run_bass_kernel_spmd

BF16 = mybir.dt.bfloat16
F32 = mybir.dt.float32
I32 = mybir.dt.int32

B, S, D = 4, 2048, 1024
N_CORES = 8
SB = 256            # query superblock rows / key gather block
N_SB = S // SB      # 8 query superblocks per core
SKL = S // 2        # local keys per core (1024)
SQH = S // 2        # own-half queries per core (1024)
MASK_NEG = -1.0e5
GROUPS = [[0, 1, 2, 3, 4, 5, 6, 7]]
ND = D // 128       # 8 d-tiles
NO = D // 128       # 8 o-tiles
NSKL = SKL // 128   # 8 local key tiles


def _legalize_waits(nc):
    """Split multi-wait instructions into single-wait NOP chains.

    The walrus here accepts at most one sync-wait command per instruction,
    while TileContext emits several `on_wait` entries on one instruction.
    Hoist all but the last wait onto same-engine NOPs placed immediately
    before the instruction; the engine sequencer stalls on each in order.
    """
    uid = 0
    for fn in nc.m.functions:
        for bb in fn.blocks:
            out = []
            for inst in bb.instructions:
                si = inst.sync_info
                waits = list(si.on_wait) if si and si.on_wait else []
                if len(waits) > 1:
                    for w in waits[:-1]:
                        nop = mybir.InstNoOp(name=f"waitsplit_{uid}", ins=[], outs=[])
                        uid += 1
                        nop.engine = inst.engine
                        si2 = copy.deepcopy(si)
                        si2.on_wait = [w]
                        si2.on_update = []
                        nop.sync_info = si2
                        out.append(nop)
                    si.on_wait = waits[-1:]
                    inst.sync_info = si
                out.append(inst)
            bb.instructions = out


def _emit_warmup(nc, tc, warm_pool):
    # HAM warmup: dependency-free matmuls keep PE busy during the initial
    # DMA wait (rep 0 only -- in steady state the PE never idles long
    # enough to re-gate the clock)
    wsrc = warm_pool.tile([128, 512], BF16, tag="wsrc", name="wsrc")
    nc.gpsimd.memset(wsrc[:], 0.0)
    with tc.tile_pool(name="wps", bufs=1, space="PSUM") as wps_pool:
        wps = wps_pool.tile([128, 512], F32, tag="wps", name="wps")
        for i in range(19):
            nc.tensor.matmul(
                wps[:], wsrc[:, 0:128], wsrc[:], start=(i == 0), stop=(i == 18)
            )


def _emit_const_loads(nc, p, t):
    # loads that are identical every rep: weights, mask, gather indices,
    # ones (kept resident across reps)
    t["wq"], t["wk"], t["wv"] = [], [], []
    for nm, lst, dram in (("wq", t["wq"], p["wqT_d"]), ("wk", t["wk"], p["wkT_d"]),
                          ("wv", t["wv"], p["wvT_d"])):
        for i in range(ND):
            w = p["w_pool"].tile([128, D], BF16, tag="w", name=f"{nm}{i}")
            nc.sync.dma_start(w[:], dram[i * 128:(i + 1) * 128, :])
            lst.append(w)
    t["mask"] = p["mask_pool"].tile([128, SB], F32, tag="mask", name="mask0")
    nc.sync.dma_start(t["mask"][:], p["mask_d"][:])
    t["idx"] = p["idx_pool"].tile([128, 2], I32, tag="idx", name="idx0")
    nc.sync.dma_start(t["idx"][:], p["qidx_d"][:])
    t["ones"] = p["ones_pool"].tile([128, 1], BF16, tag="ones", name="ones0")
    nc.gpsimd.memset(t["ones"][:], 1.0)


def _emit_x_loads(nc, p, rep):
    xq_t = []
    for i in range(ND):
        x = p["xqT_pool"].tile([128, SQH], BF16, tag="xqT", name=f"xq{rep}_{i}")
        nc.sync.dma_start(x[:, 0:512], p["xqT_d"][i * 128:(i + 1) * 128, 0:512])
        xq_t.append(x)
    for i in range(ND):
        nc.sync.dma_start(
            xq_t[i][:, 512:SQH], p["xqT_d"][i * 128:(i + 1) * 128, 512:SQH]
        )
    xk_t = []
    for i in range(ND):
        x = p["xkT_pool"].tile([128, SKL], BF16, tag="xkT", name=f"xk{rep}_{i}")
        nc.sync.dma_start(x[:], p["xkT_d"][i * 128:(i + 1) * 128, :])
        xk_t.append(x)
    return xq_t, xk_t


def _emit_qproj_exchange(nc, p, t, psum1, xq_t, rep):
    # Q^T-half[o, sq_own] = sum_d wqT[d, o] * xqT[d, sq_own], staged to qout
    # with permuted rows: psum partition pp of o-tile ot -> row 8*pp + ot
    # (so a 16KB gather row holds one o-dim from each of the 8 o-tiles)
    qout = p["dram_pool"].tile([D, SQH], BF16, tag="qout", name=f"qout{rep}", bufs=1)
    qg = p["dram_pool"].tile([D, 8 * SQH], BF16, tag="qg", name=f"qg{rep}",
                             bufs=1, addr_space="Shared")
    for sqc in range(SQH // 512):
        for ot in range(NO):
            ps = psum1.tile([128, 512], F32, tag="ps1", name=f"psq{rep}_{sqc}_{ot}")
            for d in range(ND):
                nc.tensor.matmul(
                    ps[:],
                    t["wq"][d][:, ot * 128:(ot + 1) * 128],
                    xq_t[d][:, sqc * 512:(sqc + 1) * 512],
                    start=(d == 0),
                    stop=(d == ND - 1),
                )
            qs = p["qs_pool"].tile([128, 512], BF16, tag="qs",
                                   name=f"qs{rep}_{sqc}_{ot}")
            nc.vector.tensor_copy(qs[:], ps[:])
            nc.sync.dma_start(
                qout[ot:ot + 8 * 127 + 1:8, sqc * 512:(sqc + 1) * 512], qs[:]
            )
    # exchange: 8-core AllGather of the Q^T halves, then pull this pair's
    # two slabs back via indirect gather (collective/DMA silicon only --
    # overlaps the K/V projections on the PE)
    nc.gpsimd.collective_compute(
        "AllGather", mybir.AluOpType.bypass, replica_groups=GROUPS,
        ins=[qout[:]], outs=[qg[:]],
    )
    QG_t = []
    for sl in range(2):
        g = p["QG_pool"].tile([128, 8 * SQH], BF16, tag=f"QG{sl}", bufs=1,
                              name=f"QG{rep}_{sl}")
        nc.gpsimd.indirect_dma_start(
            out=g[:],
            out_offset=None,
            in_=qg[:],
            in_offset=bass.IndirectOffsetOnAxis(ap=t["idx"][:, sl:sl + 1], axis=0),
        )
        QG_t.append(g)
    return QG_t


def _emit_kv_proj(nc, p, t, psum1, xk_t, rep):
    # K^T[o, skl] = sum_d wkT[d, o] * xkT[d, skl]
    KT_t = [
        p["KT_pool"].tile([128, SKL], BF16, tag="KT", name=f"KT{rep}_{ot}")
        for ot in range(NO)
    ]
    for skc in range(SKL // 512):
        for ot in range(NO):
            ps = psum1.tile([128, 512], F32, tag="ps1", name=f"psk{rep}_{skc}_{ot}")
            for d in range(ND):
                nc.tensor.matmul(
                    ps[:],
                    t["wk"][d][:, ot * 128:(ot + 1) * 128],
                    xk_t[d][:, skc * 512:(skc + 1) * 512],
                    start=(d == 0),
                    stop=(d == ND - 1),
                )
            nc.vector.tensor_copy(KT_t[ot][:, skc * 512:(skc + 1) * 512], ps[:])
    # V[skl, o] = sum_d xkT[d, skl] * wvT[d, o]
    V_t = [
        p["V_pool"].tile([128, D], BF16, tag="V", name=f"V{rep}_{st}")
        for st in range(NSKL)
    ]
    for st in range(NSKL):
        for oc in range(D // 512):
            ps = psum1.tile([128, 512], F32, tag="ps1", name=f"psv{rep}_{st}_{oc}")
            for d in range(ND):
                nc.tensor.matmul(
                    ps[:],
                    xk_t[d][:, st * 128:(st + 1) * 128],
                    t["wv"][d][:, oc * 512:(oc + 1) * 512],
                    start=(d == 0),
                    stop=(d == ND - 1),
                )
            nc.vector.tensor_copy(V_t[st][:, oc * 512:(oc + 1) * 512], ps[:])
    return KT_t, V_t


def _emit_scores(nc, p, t, pss_pool, KT_t, QG_t, rep):
    # Pass A, query-chunk-major: 512-wide query chunks ascending, key tiles
    # inner; chunk c only needs gathered slab c//2. Mask + exp into
    # resident es tiles.
    es_t = []
    for kt in range(NSKL):
        w = S - SB * kt         # queries [SB*kt, S)
        es = p["es_pool"].tile([128, w], BF16, tag=f"es{kt}", bufs=1,
                               name=f"es{rep}_{kt}")
        es_t.append(es)
    for c in range(S // 512):
        sl = c // 2             # gathered slab (query half)
        for kt in range(min(2 * c + 1, NSKL - 1) + 1):
            lo = max(512 * c, 256 * kt)
            w = 512 * (c + 1) - lo
            ql = lo - SQH * sl  # column within the slab
            ps_s = pss_pool.tile([128, 512], F32, tag="pss",
                                 name=f"pss{rep}_{c}_{kt}")
            for od in range(NO):
                nc.tensor.matmul(
                    ps_s[:, 0:w],
                    KT_t[od][:, kt * 128:(kt + 1) * 128],
                    QG_t[sl][:, SQH * od + ql:SQH * od + ql + w],
                    start=(od == 0),
                    stop=(od == NO - 1),
                )
            if c == kt // 2:
                # first chunk for this key tile: its first 256 columns are
                # the diagonal superblock m == kt
                nc.vector.tensor_add(ps_s[:, 0:SB], ps_s[:, 0:SB], t["mask"][:])
            nc.scalar.activation(
                es_t[kt][:, lo - SB * kt:lo - SB * kt + w],
                ps_s[:, 0:w],
                mybir.ActivationFunctionType.Exp,
                scale=1.0 / 32.0,
            )
    return es_t


def _emit_av(nc, p, t, av_pool, den_pool, es_t, V_t, rep):
    # Pass B, query-stationary: per superblock, accumulate AV and the
    # softmax denominator (PE ones-matmul) in PSUM over tiles 0..m,
    # evacuate av as bf16 + den as fp32.
    for m in range(N_SB):
        n = m + 1              # local sk-tiles this superblock
        avs = [
            av_pool.tile([128, 512], F32, tag="av", name=f"av{rep}_{m}_{i}")
            for i in range(4)
        ]
        dens = [
            den_pool.tile([128, 1], F32, tag="den", name=f"den{rep}_{m}_{i}")
            for i in range(2)
        ]
        # tile-major: finish each av tile's accumulation before starting the
        # next, so its evacuation overlaps the remaining AV matmuls instead
        # of serializing after the superblock's last matmul
        for sqh in range(2):
            for kt in range(n):
                q0 = SB * (m - kt)
                nc.tensor.matmul(
                    dens[sqh][:],
                    es_t[kt][:, q0 + sqh * 128:q0 + (sqh + 1) * 128],
                    t["ones"][:],
                    start=(kt == 0),
                    stop=(kt == n - 1),
                )
        for sqh in range(2):
            for oh in range(2):
                for kt in range(n):
                    q0 = SB * (m - kt)
                    nc.tensor.matmul(
                        avs[sqh * 2 + oh][:],
                        es_t[kt][:, q0 + sqh * 128:q0 + (sqh + 1) * 128],
                        V_t[kt][:, oh * 512:(oh + 1) * 512],
                        start=(kt == 0),
                        stop=(kt == n - 1),
                    )
        for sqh in range(2):
            r0 = m * SB + sqh * 128
            oden = p["oden_pool"].tile([128, 1], F32, tag="oden",
                                       name=f"oden{rep}_{m}_{sqh}")
            nc.vector.tensor_copy(oden[:], dens[sqh][:])
            nc.sync.dma_start(p["yden_d"][r0:r0 + 128, :], oden[:])
            oav = p["oav_pool"].tile([128, D], BF16, tag="oav",
                                     name=f"oav{rep}_{m}_{sqh}")
            for oh in range(2):
                nc.vector.tensor_copy(
                    oav[:, oh * 512:(oh + 1) * 512], avs[sqh * 2 + oh][:]
                )
                if m == N_SB - 1:
                    # kernel tail: ship each half as soon as its copy lands
                    nc.sync.dma_start(
                        p["yav_d"][r0:r0 + 128, oh * 512:(oh + 1) * 512],
                        oav[:, oh * 512:(oh + 1) * 512],
                    )
            if m != N_SB - 1:
                nc.sync.dma_start(p["yav_d"][r0:r0 + 128, :], oav[:])


def _emit_rep(nc, tc, p, t, rep):
    if rep == 0:
        _emit_warmup(nc, tc, p["warm_pool"])
        _emit_const_loads(nc, p, t)
    xq_t, xk_t = _emit_x_loads(nc, p, rep)
    with tc.tile_pool(name="psum1", bufs=4, space="PSUM") as psum1:
        QG_t = _emit_qproj_exchange(nc, p, t, psum1, xq_t, rep)
        KT_t, V_t = _emit_kv_proj(nc, p, t, psum1, xk_t, rep)
    with (
        tc.tile_pool(name="av", bufs=4, space="PSUM") as av_pool,
        tc.tile_pool(name="pss", bufs=2, space="PSUM") as pss_pool,
        tc.tile_pool(name="den", bufs=2, space="PSUM") as den_pool,
    ):
        es_t = _emit_scores(nc, p, t, pss_pool, KT_t, QG_t, rep)
        _emit_av(nc, p, t, av_pool, den_pool, es_t, V_t, rep)


def build_nc(reps=1):
    nc = bass.Bass("TRN2", target_bir_lowering=False, debug=False, num_devices=N_CORES)

    p = {
        "xqT_d": nc.dram_tensor("xqT", [D, SQH], BF16, kind="ExternalInput"),
        "xkT_d": nc.dram_tensor("xkT", [D, SKL], BF16, kind="ExternalInput"),
        "wqT_d": nc.dram_tensor("wqT", [D, D], BF16, kind="ExternalInput"),
        "wkT_d": nc.dram_tensor("wkT", [D, D], BF16, kind="ExternalInput"),
        "wvT_d": nc.dram_tensor("wvT", [D, D], BF16, kind="ExternalInput"),
        # additive causal mask for the last 128 local keys of a superblock,
        # [128 keys, 256 queries]; data depends only on h
        "mask_d": nc.dram_tensor("maskT", [128, SB], F32, kind="ExternalInput"),
        # gather view-row indices: column l = 128*(2b+l) + arange(128)
        "qidx_d": nc.dram_tensor("qidx", [128, 2], I32, kind="ExternalInput"),
        "yav_d": nc.dram_tensor("yav", [S, D], BF16, kind="ExternalOutput"),
        "yden_d": nc.dram_tensor("yden", [S, 1], F32, kind="ExternalOutput"),
    }

    with tile.TileContext(nc) as tc:
        with (
            tc.tile_pool(name="xqT", bufs=ND) as xqT_pool,
            tc.tile_pool(name="xkT", bufs=ND) as xkT_pool,
            tc.tile_pool(name="w", bufs=3 * ND) as w_pool,
            tc.tile_pool(name="KT", bufs=NO) as KT_pool,
            tc.tile_pool(name="V", bufs=NSKL) as V_pool,
            tc.tile_pool(name="QG", bufs=2) as QG_pool,
            tc.tile_pool(name="mask", bufs=1) as mask_pool,
            tc.tile_pool(name="idx", bufs=1) as idx_pool,
            tc.tile_pool(name="ones", bufs=1) as ones_pool,
            tc.tile_pool(name="qs", bufs=3) as qs_pool,
            tc.tile_pool(name="es", bufs=3) as es_pool,
            tc.tile_pool(name="oav", bufs=6) as oav_pool,
            tc.tile_pool(name="oden", bufs=4) as oden_pool,
            tc.tile_pool(name="warm", bufs=1) as warm_pool,
            tc.tile_pool(name="dram", bufs=2, space="DRAM") as dram_pool,
        ):
            p.update(
                xqT_pool=xqT_pool, xkT_pool=xkT_pool, w_pool=w_pool,
                KT_pool=KT_pool, V_pool=V_pool, QG_pool=QG_pool,
                mask_pool=mask_pool, idx_pool=idx_pool, ones_pool=ones_pool,
                qs_pool=qs_pool, es_pool=es_pool, oav_pool=oav_pool,
                oden_pool=oden_pool, warm_pool=warm_pool, dram_pool=dram_pool,
            )
            t = {}
            for rep in range(reps):
                _emit_rep(nc, tc, p, t, rep)

    _legalize_waits(nc)
    return nc


_NC_CACHE = None


def _get_nc():
    global _NC_CACHE
    if _NC_CACHE is None:
        _NC_CACHE = build_nc()
    return _NC_CACHE


def _prep_core_inputs(x, wqT, wkT, wvT, b, h):
    xb = np.ascontiguousarray(x[b])                       # [S, D] fp32
    xT = np.ascontiguousarray(xb.T).astype(ml_dtypes.bfloat16)
    xqT = np.ascontiguousarray(xT[:, SQH * h:SQH * (h + 1)])
    # keys interleave at 128-row granularity: core h owns global 128-blocks
    # {h, h+2, ..., h+14}; superblock m's local extent is exactly 128*(m+1)
    # for both cores, and only the last local key tile needs masking.
    blocks = [h + 2 * i for i in range(8)]
    xk = np.concatenate([xb[128 * t:128 * (t + 1)] for t in blocks], axis=0)
    xkT = np.ascontiguousarray(xk.T).astype(ml_dtypes.bfloat16)
    # mask for the last local key tile (global block 2m+h vs queries of
    # superblock m): keep iff 128*h + r_k <= r_q
    kk = np.arange(128)[:, None]
    qq = np.arange(SB)[None, :]
    maskT = np.where(128 * h + kk <= qq, 0.0, MASK_NEG).astype(np.float32)
    # gather rows (16KB view rows of the AllGather output): slab l of this
    # pair sits at view rows [128*(2b+l), 128*(2b+l)+128)
    qidx = np.zeros((128, 2), np.int32)
    for sl in range(2):
        qidx[:, sl] = 128 * (2 * b + sl) + np.arange(128)
    return {
        "xqT": xqT, "xkT": xkT,
        "wqT": wqT, "wkT": wkT, "wvT": wvT,
        "maskT": maskT, "qidx": qidx,
    }


def kernel(x, W_q, W_k, W_v):
    x = np.asarray(x, dtype=np.float32)
    wqT = np.ascontiguousarray(np.asarray(W_q, np.float32).T).astype(ml_dtypes.bfloat16)
    wkT = np.ascontiguousarray(np.asarray(W_k, np.float32).T).astype(ml_dtypes.bfloat16)
    wvT = np.ascontiguousarray(np.asarray(W_v, np.float32).T).astype(ml_dtypes.bfloat16)

    in_maps = []
    for c in range(N_CORES):
        b, h = divmod(c, 2)
        in_maps.append(_prep_core_inputs(x, wqT, wkT, wvT, b, h))

    nc = _get_nc()
    res = run_bass_kernel_spmd(nc, in_maps, list(range(N_CORES)))

    out = np.empty((B, S, D), dtype=np.float32)
    for b in range(B):
        av0 = np.asarray(res.results[2 * b]["yav"], dtype=np.float32)
        av1 = np.asarray(res.results[2 * b + 1]["yav"], dtype=np.float32)
        den = res.results[2 * b]["yden"] + res.results[2 * b + 1]["yden"]
        out[b] = (av0 + av1) / den
    return out
